# revision 21
# baseline (speedup 1.0000x reference)
"""Trainium2 Bass kernel for causal GQA self-attention (B=2,S=2048,D=1024,H=16,HKV=4,HD=64).

Sharding: 8 cores = DP(2 over batch) x TP(4 over GQA groups).
Each core computes, for one batch element and one GQA group (4 q heads + 1 kv head),
the partial output  y_group @ Wo[:, group_cols].T  (row-sharded Wo).
Host sums the 4 TP partials per batch element.

v4: single shared PSUM pool (4 slots x 2 banks) across all phases; phase-1
projection/rope streamed per 512-column chunk and attention blocks emitted as
soon as their inputs exist, so the whole kernel is one continuous pipeline.
N=512 attention matmuls (head pairs packed in columns), ScalarE reserved for
softmax exp (+ rms Sqrt), GpSimd does the denominator broadcast.
"""

import sys
from contextlib import ExitStack

sys.path.insert(0, "/opt/trn_rl_repo")

import numpy as np
import ml_dtypes

import concourse.bass as bass
import concourse.bacc as bacc
import concourse.tile as tile
import concourse.mybir as mybir
from concourse.bass_utils import run_bass_kernel_spmd

BF16 = mybir.dt.bfloat16
F32 = mybir.dt.float32
AF = mybir.ActivationFunctionType
BF16NP = ml_dtypes.bfloat16

D, H, HKV, HD, B, S = 1024, 16, 4, 64, 2, 2048
HG = 4              # q heads per core
KV_DIM = HKV * HD   # 256
E = HG * HD         # 256 local q-proj dim
ROPE_BASE = 10000.0
EPS = float(np.finfo(np.float32).eps)

NK = D // 128       # 8 contraction tiles for qkv projections
SQB = 256           # sq block size in attention
NB = S // SQB       # 8 blocks
NJ = S // 128       # 16 sk tiles
NS5 = S // 512      # 4 n-tiles of 512 in projections

# const block column offsets (bf16 [128, CW])
_CO_COS = 0
_CO_SIN = _CO_COS + S
_CO_NSIN = _CO_SIN + S
_CO_M0 = _CO_NSIN + S
_CO_M1 = _CO_M0 + HG * SQB
_CO_ID = _CO_M1 + HG * SQB
_CO_SEL = _CO_ID + 128          # sel4 [128,4]
_CO_BSEL = _CO_SEL + 4          # bsel4 [4,128]
_CO_O64C = _CO_BSEL + 128       # ones64col [64,1]
_CO_O64R = _CO_O64C + 1         # ones64 row [1,64]
CW = _CO_O64R + 64


def _consts():
    """Constant block baked into the NEFF (same for every core): [128, CW] bf16."""
    blk = np.zeros((128, CW), dtype=BF16NP)
    i = np.arange(32, dtype=np.float64)
    inv_freq = 1.0 / (ROPE_BASE ** (2.0 * i / HD))
    pos = np.arange(S, dtype=np.float64)
    fr = pos[:, None] * inv_freq[None, :]          # [S, 32]
    cosT = np.cos(fr).T.astype(np.float32)          # [32, S]
    sinT = np.sin(fr).T.astype(np.float32)
    blk[:, _CO_COS:_CO_COS + S] = np.tile(cosT, (4, 1)).astype(BF16NP)
    blk[:, _CO_SIN:_CO_SIN + S] = np.tile(sinT, (4, 1)).astype(BF16NP)
    blk[:, _CO_NSIN:_CO_NSIN + S] = (-np.tile(sinT, (4, 1))).astype(BF16NP)

    # causal masks for diagonal sk-tiles: pattern p in {0,1}
    # valid iff c >= 128*p + r   (r: sk row 0..127, c: sq col 0..255)
    r = np.arange(128)[:, None]
    c = np.arange(SQB)[None, :]
    for p, co in ((0, _CO_M0), (1, _CO_M1)):
        m = (c >= 128 * p + r).astype(BF16NP)       # [128, 256]
        blk[:, co:co + HG * SQB] = np.tile(m, (1, HG))

    blk[:, _CO_ID:_CO_ID + 128] = np.eye(128, dtype=BF16NP)
    sel4 = np.zeros((128, 4), dtype=BF16NP)         # sumsq selector: tops of head h
    for h in range(4):
        sel4[32 * h:32 * h + 32, h] = 1.0
    blk[:, _CO_SEL:_CO_SEL + 4] = sel4
    bsel4 = np.zeros((4, 128), dtype=BF16NP)        # broadcast f[h] -> rows 32h..32h+32
    for h in range(4):
        bsel4[h, 32 * h:32 * h + 32] = 1.0
    blk[0:4, _CO_BSEL:_CO_BSEL + 128] = bsel4
    blk[0:64, _CO_O64C] = 1.0                       # ones64col [64,1]
    blk[0:1, _CO_O64R:_CO_O64R + 64] = 1.0          # ones64 row [1,64]
    return blk


def _build():
    nc = bacc.Bacc("TRN2", debug=False)

    xt_d = nc.dram_tensor("xt", [128, NK * S], BF16, kind="ExternalInput")
    wq_d = nc.dram_tensor("wq", [128, NK * E], BF16, kind="ExternalInput")
    wkv_d = nc.dram_tensor("wkv", [128, NK * 128], BF16, kind="ExternalInput")
    wo_d = nc.dram_tensor("wo", [128, 2 * D], BF16, kind="ExternalInput")
    qg8_d = nc.dram_tensor("qg8", [4, 1], F32, kind="ExternalInput")
    out_d = nc.dram_tensor("out", [S, D], BF16, kind="ExternalOutput")

    cblk_d = nc.inline_tensor(_consts(), "cblk")

    with tile.TileContext(nc) as tc, ExitStack() as ctx:
        sp = ctx.enter_context(tc.tile_pool(name="static", bufs=1))

        def stile(shape, dt, tag):
            return sp.tile(shape, dt, name=tag, tag=tag)

        # ---- static SBUF tensors ----
        xt = stile([128, NK * S], BF16, "xt")
        wq = stile([128, NK * E], BF16, "wq")
        wkv = stile([128, NK * 128], BF16, "wkv")
        wo = stile([128, 2 * D], BF16, "wo")
        cb = stile([128, CW], BF16, "cb")
        qg8_s = stile([4, 1], F32, "qg8")
        epsb = stile([128, 1], F32, "epsb")
        zb = stile([128, 1], F32, "zb")

        # const views
        cos4 = cb[:, _CO_COS:_CO_COS + S]
        sin4 = cb[:, _CO_SIN:_CO_SIN + S]
        nsin4 = cb[:, _CO_NSIN:_CO_NSIN + S]
        mask_s = [cb[:, _CO_M0:_CO_M0 + HG * SQB], cb[:, _CO_M1:_CO_M1 + HG * SQB]]
        id128 = cb[:, _CO_ID:_CO_ID + 128]
        sel4 = cb[:, _CO_SEL:_CO_SEL + 4]
        bsel4 = cb[0:4, _CO_BSEL:_CO_BSEL + 128]
        ones64col = cb[0:64, _CO_O64C:_CO_O64C + 1]
        ones64row = cb[0:1, _CO_O64R:_CO_O64R + 64]

        qsb = [stile([128, S], BF16, f"qsb{m}") for m in range(2)]   # T/B packed
        kvsb = stile([128, S], BF16, "kvsb")                          # k(0:64) | v(64:128)
        sqq = [stile([128, S], BF16, f"sqq{m}") for m in range(2)]
        sqkv = stile([64, S], BF16, "sqkv")
        fq = stile([4, S], BF16, "fq")
        fk = stile([1, S], BF16, "fk")
        fbcq = stile([128, S], BF16, "fbcq")
        fbck = stile([64, S], BF16, "fbck")
        qr = [stile([128, S], BF16, f"qr{m}") for m in range(2)]      # rotated T/B
        kr = [stile([32, S], BF16, f"kr{m}") for m in range(2)]
        kb0 = stile([32, S], BF16, "kb0")
        qeo = stile([128, NB, 2, SQB], BF16, "qeo")   # [he|ho] x per-b [pair0|pair1]
        kdup = stile([128, S], BF16, "kdup")
        vsb = stile([128, NJ, 65], BF16, "vsb")       # [v | ones]
        yn = [stile([128, S], BF16, f"yn{m}") for m in range(2)]      # normalized y^T

        # ---- load everything (xt on the sync HWDGE ring; consts/weights on the
        # scalar ring so the two streams transfer concurrently) ----
        nc.sync.dma_start(wq[:], wq_d[:])
        nc.sync.dma_start(wkv[:], wkv_d[:])
        nc.sync.dma_start(qg8_s[:], qg8_d[:])
        for kc in range(4):
            lsl = slice(kc * 2 * S, (kc + 1) * 2 * S)
            nc.sync.dma_start(xt[:, lsl], xt_d[:, lsl])
        nc.scalar.dma_start(cb[:], cblk_d[:])
        nc.scalar.dma_start(wo[:], wo_d[:])
        nc.vector.memset(vsb[:], 1.0)  # ones column at [:, j, 64]; 0:64 overwritten below
        nc.vector.memset(epsb[:], EPS)
        nc.vector.memset(zb[:], 0.0)

        with (
            tc.tile_pool(name="pz", bufs=4, space=bass.MemorySpace.PSUM) as pz,
            tc.tile_pool(name="lns", bufs=2) as lns,
            tc.tile_pool(name="rt", bufs=2) as rt,
            tc.tile_pool(name="pa", bufs=3) as pa,
            tc.tile_pool(name="pn", bufs=1) as pn,
            tc.tile_pool(name="ob", bufs=2) as ob,
        ):
            def ztile(shape=(128, 1024), dt=F32):
                return pz.tile(list(shape), dt, name="pz", tag="pz")

            # ---- phase 1 chunk: projections + rms factors + rope + assembly
            # for s-columns 512n..512n+512 ----
            def p1_chunk(n):
                sl = slice(512 * n, 512 * (n + 1))
                pq01 = ztile()                 # m0: cols 0:512, m1: cols 512:1024
                pk8 = ztile()                  # pkv: cols 0:512; v-transposes in bank B
                for k in range(NK):
                    xsl = xt[:, k * S + 512 * n:k * S + 512 * (n + 1)]
                    st_, sp_ = (k == 0), (k == NK - 1)
                    nc.tensor.matmul(pq01[:, 0:512], wq[:, k * E:k * E + 128],
                                     xsl, start=st_, stop=sp_)
                    nc.tensor.matmul(pq01[:, 512:1024],
                                     wq[:, k * E + 128:k * E + 256],
                                     xsl, start=st_, stop=sp_, skip_group_check=True)
                    nc.tensor.matmul(pk8[:, 0:512], wkv[:, k * 128:(k + 1) * 128],
                                     xsl, start=st_, stop=sp_)
                for m in range(2):
                    nc.vector.tensor_copy(qsb[m][:, sl], pq01[:, 512 * m:512 * (m + 1)])
                    nc.vector.tensor_mul(sqq[m][:, sl], qsb[m][:, sl], qsb[m][:, sl])
                nc.vector.tensor_copy(kvsb[:, sl], pk8[:, 0:512])
                nc.vector.tensor_mul(sqkv[:, sl], kvsb[0:64, sl], kvsb[0:64, sl])
                nc.sync.dma_start(kb0[:, sl], kvsb[32:64, sl])
                # v transpose: [64,128] slices -> [128,64] (into pk8 bank B)
                for t in range(4):
                    st_ = 4 * n + t
                    ptr = pk8[:, 512 + 32 * t:512 + 32 * (t + 1)].bitcast(BF16)
                    nc.tensor.transpose(
                        ptr, kvsb[64:128, 128 * st_:128 * (st_ + 1)],
                        id128[64:128, 64:128])
                    nc.vector.tensor_copy(vsb[:, st_, 0:64], ptr)

                # rms factors: f = gain/8 * (ssq/HD + eps)^-1/2 (Sqrt + fast recip)
                pf = ztile((33, 1024))         # psq rows 0:4 bank A; psk row 32 bank B
                psq = pf[0:4, 0:512]
                psk = pf[32:33, 512:1024]
                nc.tensor.matmul(psq, sel4, sqq[0][:, sl], start=True, stop=False)
                nc.tensor.matmul(psq, sel4, sqq[1][:, sl], start=False, stop=True)
                nc.tensor.matmul(psk, ones64col, sqkv[:, sl], start=True, stop=True,
                                 skip_group_check=True)
                fsq = lns.tile([4, 512], F32, name="fsq", tag="fsq")
                nc.scalar.activation(fsq[:], psq, AF.Sqrt, scale=1.0 / HD,
                                     bias=epsb[0:4, :])
                frq = lns.tile([4, 512], F32, name="frq", tag="frq")
                nc.vector.reciprocal_approx_fast(frq[:], fsq[:])
                nc.vector.tensor_scalar_mul(fq[:, sl], frq[:], qg8_s[:, :])
                fsk = lns.tile([1, 512], F32, name="fsk", tag="fsk")
                nc.scalar.activation(fsk[:], psk, AF.Sqrt, scale=1.0 / HD,
                                     bias=epsb[0:1, :])
                frk = lns.tile([1, 512], F32, name="frk", tag="frk")
                nc.vector.reciprocal_approx_fast(frk[:], fsk[:])
                nc.vector.tensor_scalar_mul(fk[:, sl], frk[:], 1.0)
                # broadcast factors along hd rows via PE
                pbx = ztile()                  # pb cols 0:512; pbk cols 512:1024
                nc.tensor.matmul(pbx[:, 0:512], bsel4, fq[:, sl],
                                 start=True, stop=True)
                nc.vector.tensor_copy(fbcq[:, sl], pbx[:, 0:512])
                nc.tensor.matmul(pbx[0:64, 512:1024], ones64row, fk[:, sl],
                                 start=True, stop=True, skip_group_check=True)
                nc.vector.tensor_copy(fbck[:, sl], pbx[0:64, 512:1024])

                # rope + scale (DVE, bf16)
                t1 = rt.tile([128, 512], BF16, name="t1", tag="t1")
                t2 = rt.tile([128, 512], BF16, name="t2", tag="t2")
                nc.vector.tensor_mul(t1[:], qsb[0][:, sl], cos4[:, sl])
                nc.vector.tensor_mul(t2[:], qsb[1][:, sl], sin4[:, sl])
                nc.vector.tensor_add(t1[:], t1[:], t2[:])
                nc.vector.tensor_mul(qr[0][:, sl], t1[:], fbcq[:, sl])
                u1 = rt.tile([128, 512], BF16, name="t1", tag="t1")
                u2 = rt.tile([128, 512], BF16, name="t2", tag="t2")
                nc.vector.tensor_mul(u1[:], qsb[0][:, sl], nsin4[:, sl])
                nc.vector.tensor_mul(u2[:], qsb[1][:, sl], cos4[:, sl])
                nc.vector.tensor_add(u1[:], u1[:], u2[:])
                nc.vector.tensor_mul(qr[1][:, sl], u1[:], fbcq[:, sl])
                k1 = rt.tile([32, 512], BF16, name="k1", tag="k1")
                k2 = rt.tile([32, 512], BF16, name="k2", tag="k2")
                nc.vector.tensor_mul(k1[:], kvsb[0:32, sl], cos4[0:32, sl])
                nc.vector.tensor_mul(k2[:], kb0[:, sl], sin4[0:32, sl])
                nc.vector.tensor_add(k1[:], k1[:], k2[:])
                nc.vector.tensor_mul(kr[0][:, sl], k1[:], fbck[0:32, sl])
                k3 = rt.tile([32, 512], BF16, name="k1", tag="k1")
                k4 = rt.tile([32, 512], BF16, name="k2", tag="k2")
                nc.vector.tensor_mul(k3[:], kvsb[0:32, sl], nsin4[0:32, sl])
                nc.vector.tensor_mul(k4[:], kb0[:, sl], cos4[0:32, sl])
                nc.vector.tensor_add(k3[:], k3[:], k4[:])
                nc.vector.tensor_mul(kr[1][:, sl], k3[:], fbck[0:32, sl])

                # layout assembly (DMA partition/column moves) for this chunk
                bh = slice(2 * n, 2 * (n + 1))
                for h, (rbase, pcol) in enumerate(((0, 0), (64, 0), (0, 1), (64, 1))):
                    src0 = qr[0][32 * h:32 * h + 32, sl].rearrange(
                        "p (b s) -> p b s", b=2)
                    src1 = qr[1][32 * h:32 * h + 32, sl].rearrange(
                        "p (b s) -> p b s", b=2)
                    nc.sync.dma_start(qeo[rbase:rbase + 32, bh, pcol, :], src0)
                    nc.sync.dma_start(qeo[rbase + 32:rbase + 64, bh, pcol, :], src1)
                nc.sync.dma_start(kdup[0:32, sl], kr[0][:, sl])
                nc.sync.dma_start(kdup[32:64, sl], kr[1][:, sl])
                nc.sync.dma_start(kdup[64:96, sl], kr[0][:, sl])
                nc.sync.dma_start(kdup[96:128, sl], kr[1][:, sl])

            # ---- attention + interleaved output projection for q-block b ----
            def attn_block(b):
                sq = slice(SQB * b, SQB * (b + 1))
                jmax = 2 * b + 1
                yt = pz.tile([65, 1024], F32, name="yt", tag="pz")
                for j in range(jmax + 1):
                    st = ztile()
                    jc = slice(128 * j, 128 * (j + 1))
                    nc.tensor.matmul(st[:, 0:512], kdup[0:64, jc],
                                     qeo[0:64, b, :, :],
                                     start=True, stop=True)
                    nc.tensor.matmul(st[:, 512:1024], kdup[64:128, jc],
                                     qeo[64:128, b, :, :],
                                     start=True, stop=True, skip_group_check=True)
                    pt = pa.tile([128, 1024], BF16, name="pt", tag="pt")
                    nc.scalar.activation(pt[:], st[:], AF.Exp, bias=zb[:, :])
                    if j >= 2 * b:
                        nc.vector.tensor_mul(pt[:], pt[:], mask_s[j - 2 * b])
                    nc.tensor.matmul(yt[:, 0:512], vsb[:, j, :], pt[:, 0:512],
                                     start=(j == 0), stop=(j == jmax))
                    nc.tensor.matmul(yt[:, 512:1024], vsb[:, j, :], pt[:, 512:1024],
                                     start=(j == 0), stop=(j == jmax),
                                     skip_group_check=True)

                # denominators: yt row 64 = sum exp per (head, query)
                # (staged to a base-0 tile: partition_broadcast reads partition 0)
                dnb = pn.tile([1, 1024], F32, name="dnb", tag="dnb")
                nc.vector.tensor_copy(dnb[:], yt[64:65, :])
                rbb = pn.tile([64, 1024], F32, name="rbb", tag="rbb")
                nc.gpsimd.partition_broadcast(rbb[:], dnb[:])
                rbs = pn.tile([64, 1024], F32, name="rbs", tag="rbs")
                nc.vector.reciprocal_approx_fast(rbs[:], rbb[:])
                # normalize: yn0 rows = [h0 | h2], yn1 rows = [h1 | h3]
                # (upper halves staged at base 0 then DMA-moved: walrus requires
                # tensor_tensor dst/src start partitions to match)
                for m in range(2):
                    nc.vector.tensor_mul(yn[m][0:64, sq],
                                         yt[0:64, 512 * m:512 * m + 256],
                                         rbs[:, 512 * m:512 * m + 256])
                    stg = pa.tile([64, 256], BF16, name="stg", tag="stg", bufs=2)
                    nc.vector.tensor_mul(stg[:],
                                         yt[0:64, 512 * m + 256:512 * m + 512],
                                         rbs[:, 512 * m + 256:512 * m + 512])
                    nc.sync.dma_start(yn[m][64:128, sq], stg[:])

                # output projection for the two 128-row s-tiles of this block
                for t in (2 * b, 2 * b + 1):
                    ssl = slice(128 * t, 128 * (t + 1))
                    pot = ztile()
                    for nh in range(2):
                        nsl = slice(512 * nh, 512 * (nh + 1))
                        for kk in range(2):
                            nc.tensor.matmul(
                                pot[:, nsl], yn[kk][:, ssl],
                                wo[:, kk * D + 512 * nh:kk * D + 512 * (nh + 1)],
                                start=(kk == 0), stop=(kk == 1),
                                skip_group_check=(nh == 1))
                    ot = ob.tile([128, D], BF16, name="ot", tag="ot")
                    nc.vector.tensor_copy(ot[:], pot[:])
                    nc.sync.dma_start(out_d[ssl, :], ot[:])

            # ---- interleaved emission: each attention block right after the
            # phase-1 chunk that completes its inputs ----
            sched = {0: [0, 1], 1: [2, 3], 2: [4, 5], 3: [6, 7]}
            for n in range(NS5):
                p1_chunk(n)
                for b in sched[n]:
                    attn_block(b)

    nc.finalize()
    return nc


_NC = None


def _get_nc():
    global _NC
    if _NC is None:
        _NC = _build()
    return _NC


def _perm():
    tops = [h * 64 + i for h in range(HG) for i in range(32)]
    bots = [h * 64 + 32 + i for h in range(HG) for i in range(32)]
    return tops + bots


def build_inmaps(x, Wq, Wk, Wv, Wo, q_gain):
    x = np.asarray(x, dtype=np.float32)
    Wq = np.asarray(Wq, dtype=np.float32)
    Wk = np.asarray(Wk, dtype=np.float32)
    Wv = np.asarray(Wv, dtype=np.float32)
    Wo = np.asarray(Wo, dtype=np.float32)
    q_gain = np.asarray(q_gain, dtype=np.float32)

    perm = _perm()
    in_maps = []
    for c in range(8):
        dp, tp = divmod(c, 4)
        # xt[p, k*S+s] = x[dp][s, 128k+p]
        xt_p = np.ascontiguousarray(
            x[dp].reshape(S, NK, 128).transpose(2, 1, 0).reshape(128, NK * S)
        ).astype(BF16NP)
        wq_sel = Wq[tp * E:(tp + 1) * E].T[:, perm]          # [D, 256] permuted
        wq_p = np.ascontiguousarray(
            wq_sel.reshape(NK, 128, E).transpose(1, 0, 2).reshape(128, NK * E)
        ).astype(BF16NP)
        wk_sel = Wk[tp * HD:(tp + 1) * HD].T                  # [D, 64]
        wv_sel = Wv[tp * HD:(tp + 1) * HD].T
        wkv_sel = np.concatenate([wk_sel, wv_sel], axis=1)    # [D, 128]
        wkv_p = np.ascontiguousarray(
            wkv_sel.reshape(NK, 128, 128).transpose(1, 0, 2).reshape(128, NK * 128)
        ).astype(BF16NP)
        # wo rows ordered [h0, h2, h1, h3] to match yn stacking
        horder = [0, 2, 1, 3]
        wo_cols = np.concatenate(
            [np.arange(tp * E + h * HD, tp * E + (h + 1) * HD) for h in horder])
        wo_sel = Wo[:, wo_cols].T                             # [256, D]
        wo_p = np.ascontiguousarray(
            wo_sel.reshape(2, 128, D).transpose(1, 0, 2).reshape(128, 2 * D)
        ).astype(BF16NP)
        g = q_gain[tp * HG:(tp + 1) * HG].astype(np.float64)
        qg8 = (g / 8.0).astype(np.float32).reshape(4, 1)
        in_maps.append({
            "xt": xt_p, "wq": wq_p, "wkv": wkv_p, "wo": wo_p, "qg8": qg8,
        })
    return in_maps


def kernel(x, Wq, Wk, Wv, Wo, q_gain):
    in_maps = build_inmaps(x, Wq, Wk, Wv, Wo, q_gain)
    nc = _get_nc()
    res = run_bass_kernel_spmd(nc, in_maps, core_ids=list(range(8)))
    out = np.zeros((B, S, D), dtype=np.float32)
    for c in range(8):
        out[c // 4] += res.results[c]["out"].astype(np.float32)
    return out


# revision 23
# speedup vs baseline: 1.0397x; 1.0397x over previous
"""Trainium2 Bass kernel for causal GQA self-attention (B=2,S=2048,D=1024,H=16,HKV=4,HD=64).

Sharding: 8 cores = DP(2 over batch) x TP(4 over GQA groups).
Each core computes, for one batch element and one GQA group (4 q heads + 1 kv head),
the partial output  y_group @ Wo[:, group_cols].T  (row-sharded Wo).
Host sums the 4 TP partials per batch element.

v4: single shared PSUM pool (4 slots x 2 banks) across all phases; phase-1
projection/rope streamed per 512-column chunk and attention blocks emitted as
soon as their inputs exist, so the whole kernel is one continuous pipeline.
N=512 attention matmuls (head pairs packed in columns), ScalarE reserved for
softmax exp (+ rms Sqrt), GpSimd does the denominator broadcast.
"""

import sys
from contextlib import ExitStack

sys.path.insert(0, "/opt/trn_rl_repo")

import numpy as np
import ml_dtypes

import concourse.bass as bass
import concourse.bacc as bacc
import concourse.tile as tile
import concourse.mybir as mybir
from concourse.bass_utils import run_bass_kernel_spmd

BF16 = mybir.dt.bfloat16
F32 = mybir.dt.float32
AF = mybir.ActivationFunctionType
BF16NP = ml_dtypes.bfloat16

D, H, HKV, HD, B, S = 1024, 16, 4, 64, 2, 2048
HG = 4              # q heads per core
KV_DIM = HKV * HD   # 256
E = HG * HD         # 256 local q-proj dim
ROPE_BASE = 10000.0
EPS = float(np.finfo(np.float32).eps)

NK = D // 128       # 8 contraction tiles for qkv projections
SQB = 256           # sq block size in attention
NB = S // SQB       # 8 blocks
NJ = S // 128       # 16 sk tiles
NS5 = S // 512      # 4 n-tiles of 512 in projections

# const block column offsets (bf16 [128, CW])
_CO_COS = 0
_CO_SIN = _CO_COS + S
_CO_NSIN = _CO_SIN + S
_CO_M0 = _CO_NSIN + S
_CO_M1 = _CO_M0 + HG * SQB
_CO_ID = _CO_M1 + HG * SQB
_CO_SEL = _CO_ID + 128          # sel4 [128,4]
_CO_BSEL = _CO_SEL + 4          # bsel4 [4,128]
_CO_O64C = _CO_BSEL + 128       # ones64col [64,1]
_CO_O64R = _CO_O64C + 1         # ones64 row [1,64]
CW = _CO_O64R + 64


def _consts():
    """Constant block baked into the NEFF (same for every core): [128, CW] bf16."""
    blk = np.zeros((128, CW), dtype=BF16NP)
    i = np.arange(32, dtype=np.float64)
    inv_freq = 1.0 / (ROPE_BASE ** (2.0 * i / HD))
    pos = np.arange(S, dtype=np.float64)
    fr = pos[:, None] * inv_freq[None, :]          # [S, 32]
    cosT = np.cos(fr).T.astype(np.float32)          # [32, S]
    sinT = np.sin(fr).T.astype(np.float32)
    blk[:, _CO_COS:_CO_COS + S] = np.tile(cosT, (4, 1)).astype(BF16NP)
    blk[:, _CO_SIN:_CO_SIN + S] = np.tile(sinT, (4, 1)).astype(BF16NP)
    blk[:, _CO_NSIN:_CO_NSIN + S] = (-np.tile(sinT, (4, 1))).astype(BF16NP)

    # causal masks for diagonal sk-tiles: pattern p in {0,1}
    # valid iff c >= 128*p + r   (r: sk row 0..127, c: sq col 0..255)
    r = np.arange(128)[:, None]
    c = np.arange(SQB)[None, :]
    for p, co in ((0, _CO_M0), (1, _CO_M1)):
        m = (c >= 128 * p + r).astype(BF16NP)       # [128, 256]
        blk[:, co:co + HG * SQB] = np.tile(m, (1, HG))

    blk[:, _CO_ID:_CO_ID + 128] = np.eye(128, dtype=BF16NP)
    sel4 = np.zeros((128, 4), dtype=BF16NP)         # sumsq selector: tops of head h
    for h in range(4):
        sel4[32 * h:32 * h + 32, h] = 1.0
    blk[:, _CO_SEL:_CO_SEL + 4] = sel4
    bsel4 = np.zeros((4, 128), dtype=BF16NP)        # broadcast f[h] -> rows 32h..32h+32
    for h in range(4):
        bsel4[h, 32 * h:32 * h + 32] = 1.0
    blk[0:4, _CO_BSEL:_CO_BSEL + 128] = bsel4
    blk[0:64, _CO_O64C] = 1.0                       # ones64col [64,1]
    blk[0:1, _CO_O64R:_CO_O64R + 64] = 1.0          # ones64 row [1,64]
    return blk


def _build():
    nc = bacc.Bacc("TRN2", debug=False)

    xt_d = nc.dram_tensor("xt", [128, NK * S], BF16, kind="ExternalInput")
    wq_d = nc.dram_tensor("wq", [128, NK * E], BF16, kind="ExternalInput")
    wkv_d = nc.dram_tensor("wkv", [128, NK * 128], BF16, kind="ExternalInput")
    wo_d = nc.dram_tensor("wo", [128, 2 * D], BF16, kind="ExternalInput")
    qg8_d = nc.dram_tensor("qg8", [4, 1], F32, kind="ExternalInput")
    out_d = nc.dram_tensor("out", [S, D], BF16, kind="ExternalOutput")

    cblk_d = nc.inline_tensor(_consts(), "cblk")

    with tile.TileContext(nc) as tc, ExitStack() as ctx:
        sp = ctx.enter_context(tc.tile_pool(name="static", bufs=1))

        def stile(shape, dt, tag):
            return sp.tile(shape, dt, name=tag, tag=tag)

        # ---- static SBUF tensors ----
        xt = stile([128, NK * S], BF16, "xt")
        wq = stile([128, NK * E], BF16, "wq")
        wkv = stile([128, NK * 128], BF16, "wkv")
        wo = stile([128, 2 * D], BF16, "wo")
        cb = stile([128, CW], BF16, "cb")
        qg8_s = stile([4, 1], F32, "qg8")
        epsb = stile([128, 1], F32, "epsb")
        zb = stile([128, 1], F32, "zb")

        # const views
        cos4 = cb[:, _CO_COS:_CO_COS + S]
        sin4 = cb[:, _CO_SIN:_CO_SIN + S]
        nsin4 = cb[:, _CO_NSIN:_CO_NSIN + S]
        mask_s = [cb[:, _CO_M0:_CO_M0 + HG * SQB], cb[:, _CO_M1:_CO_M1 + HG * SQB]]
        id128 = cb[:, _CO_ID:_CO_ID + 128]
        sel4 = cb[:, _CO_SEL:_CO_SEL + 4]
        bsel4 = cb[0:4, _CO_BSEL:_CO_BSEL + 128]
        ones64col = cb[0:64, _CO_O64C:_CO_O64C + 1]
        ones64row = cb[0:1, _CO_O64R:_CO_O64R + 64]

        qsb = [stile([128, S], BF16, f"qsb{m}") for m in range(2)]   # T/B packed
        kvsb = stile([128, S], BF16, "kvsb")                          # k(0:64) | v(64:128)
        sqq = [stile([128, S], BF16, f"sqq{m}") for m in range(2)]
        sqkv = stile([64, S], BF16, "sqkv")
        fq = stile([4, S], BF16, "fq")
        fk = stile([1, S], BF16, "fk")
        fbcq = stile([128, S], BF16, "fbcq")
        fbck = stile([64, S], BF16, "fbck")
        qr = [stile([128, S], BF16, f"qr{m}") for m in range(2)]      # rotated T/B
        kr = [stile([32, S], BF16, f"kr{m}") for m in range(2)]
        kb0 = stile([32, S], BF16, "kb0")
        qeo = stile([128, NB, 2, SQB], BF16, "qeo")   # [he|ho] x per-b [pair0|pair1]
        kdup = stile([128, S], BF16, "kdup")
        vsb = stile([128, NJ, 65], BF16, "vsb")       # [v | ones]
        yn = [stile([128, S], BF16, f"yn{m}") for m in range(2)]      # normalized y^T

        # ---- load everything (xt on the sync HWDGE ring; consts/weights on the
        # scalar ring so the two streams transfer concurrently) ----
        nc.sync.dma_start(wq[:], wq_d[:])
        nc.sync.dma_start(wkv[:], wkv_d[:])
        nc.sync.dma_start(qg8_s[:], qg8_d[:])
        for kc in range(4):
            lsl = slice(kc * 2 * S, (kc + 1) * 2 * S)
            nc.sync.dma_start(xt[:, lsl], xt_d[:, lsl])
        nc.scalar.dma_start(cb[:], cblk_d[:])
        nc.scalar.dma_start(wo[:], wo_d[:])
        nc.vector.memset(vsb[:], 1.0)  # ones column at [:, j, 64]; 0:64 overwritten below
        nc.vector.memset(epsb[:], EPS)
        nc.vector.memset(zb[:], 0.0)

        with (
            tc.tile_pool(name="pz", bufs=4, space=bass.MemorySpace.PSUM) as pz,
            tc.tile_pool(name="lns", bufs=2) as lns,
            tc.tile_pool(name="rt", bufs=2) as rt,
            tc.tile_pool(name="pa", bufs=4) as pa,
            tc.tile_pool(name="pn", bufs=1) as pn,
            tc.tile_pool(name="ob", bufs=2) as ob,
        ):
            def ztile(shape=(128, 1024), dt=F32):
                return pz.tile(list(shape), dt, name="pz", tag="pz")

            # ---- phase 1 chunk: projections + rms factors + rope + assembly
            # for s-columns 512n..512n+512 ----
            def p1_chunk(n):
                sl = slice(512 * n, 512 * (n + 1))
                pq01 = ztile()                 # m0: cols 0:512, m1: cols 512:1024
                pk8 = ztile()                  # pkv: cols 0:512; v-transposes in bank B
                for k in range(NK):
                    xsl = xt[:, k * S + 512 * n:k * S + 512 * (n + 1)]
                    st_, sp_ = (k == 0), (k == NK - 1)
                    nc.tensor.matmul(pq01[:, 0:512], wq[:, k * E:k * E + 128],
                                     xsl, start=st_, stop=sp_)
                    nc.tensor.matmul(pq01[:, 512:1024],
                                     wq[:, k * E + 128:k * E + 256],
                                     xsl, start=st_, stop=sp_, skip_group_check=True)
                    nc.tensor.matmul(pk8[:, 0:512], wkv[:, k * 128:(k + 1) * 128],
                                     xsl, start=st_, stop=sp_)
                for m in range(2):
                    nc.vector.tensor_copy(qsb[m][:, sl], pq01[:, 512 * m:512 * (m + 1)])
                    nc.vector.tensor_mul(sqq[m][:, sl], qsb[m][:, sl], qsb[m][:, sl])
                nc.vector.tensor_copy(kvsb[:, sl], pk8[:, 0:512])
                nc.vector.tensor_mul(sqkv[:, sl], kvsb[0:64, sl], kvsb[0:64, sl])
                nc.sync.dma_start(kb0[:, sl], kvsb[32:64, sl])
                # v transpose: [64,128] slices -> [128,64] (into pk8 bank B)
                for t in range(4):
                    st_ = 4 * n + t
                    ptr = pk8[:, 512 + 32 * t:512 + 32 * (t + 1)].bitcast(BF16)
                    nc.tensor.transpose(
                        ptr, kvsb[64:128, 128 * st_:128 * (st_ + 1)],
                        id128[64:128, 64:128])
                    nc.vector.tensor_copy(vsb[:, st_, 0:64], ptr)

                # rms factors: f = gain/8 * (ssq/HD + eps)^-1/2 (Sqrt + fast recip)
                pf = ztile((33, 1024))         # psq rows 0:4 bank A; psk row 32 bank B
                psq = pf[0:4, 0:512]
                psk = pf[32:33, 512:1024]
                nc.tensor.matmul(psq, sel4, sqq[0][:, sl], start=True, stop=False)
                nc.tensor.matmul(psq, sel4, sqq[1][:, sl], start=False, stop=True)
                nc.tensor.matmul(psk, ones64col, sqkv[:, sl], start=True, stop=True,
                                 skip_group_check=True)
                fsq = lns.tile([4, 512], F32, name="fsq", tag="fsq")
                nc.scalar.activation(fsq[:], psq, AF.Sqrt, scale=1.0 / HD,
                                     bias=epsb[0:4, :])
                frq = lns.tile([4, 512], F32, name="frq", tag="frq")
                nc.vector.reciprocal_approx_fast(frq[:], fsq[:])
                nc.vector.tensor_scalar_mul(fq[:, sl], frq[:], qg8_s[:, :])
                fsk = lns.tile([1, 512], F32, name="fsk", tag="fsk")
                nc.scalar.activation(fsk[:], psk, AF.Sqrt, scale=1.0 / HD,
                                     bias=epsb[0:1, :])
                frk = lns.tile([1, 512], F32, name="frk", tag="frk")
                nc.vector.reciprocal_approx_fast(frk[:], fsk[:])
                nc.vector.tensor_scalar_mul(fk[:, sl], frk[:], 1.0)
                # broadcast factors along hd rows via PE
                pbx = ztile()                  # pb cols 0:512; pbk cols 512:1024
                nc.tensor.matmul(pbx[:, 0:512], bsel4, fq[:, sl],
                                 start=True, stop=True)
                nc.vector.tensor_copy(fbcq[:, sl], pbx[:, 0:512])
                nc.tensor.matmul(pbx[0:64, 512:1024], ones64row, fk[:, sl],
                                 start=True, stop=True, skip_group_check=True)
                nc.vector.tensor_copy(fbck[:, sl], pbx[0:64, 512:1024])

                # rope + scale (DVE, bf16)
                t1 = rt.tile([128, 512], BF16, name="t1", tag="t1")
                t2 = rt.tile([128, 512], BF16, name="t2", tag="t2")
                nc.vector.tensor_mul(t1[:], qsb[0][:, sl], cos4[:, sl])
                nc.vector.tensor_mul(t2[:], qsb[1][:, sl], sin4[:, sl])
                nc.vector.tensor_add(t1[:], t1[:], t2[:])
                nc.vector.tensor_mul(qr[0][:, sl], t1[:], fbcq[:, sl])
                u1 = rt.tile([128, 512], BF16, name="t1", tag="t1")
                u2 = rt.tile([128, 512], BF16, name="t2", tag="t2")
                nc.vector.tensor_mul(u1[:], qsb[0][:, sl], nsin4[:, sl])
                nc.vector.tensor_mul(u2[:], qsb[1][:, sl], cos4[:, sl])
                nc.vector.tensor_add(u1[:], u1[:], u2[:])
                nc.vector.tensor_mul(qr[1][:, sl], u1[:], fbcq[:, sl])
                k1 = rt.tile([32, 512], BF16, name="k1", tag="k1")
                k2 = rt.tile([32, 512], BF16, name="k2", tag="k2")
                nc.vector.tensor_mul(k1[:], kvsb[0:32, sl], cos4[0:32, sl])
                nc.vector.tensor_mul(k2[:], kb0[:, sl], sin4[0:32, sl])
                nc.vector.tensor_add(k1[:], k1[:], k2[:])
                nc.vector.tensor_mul(kr[0][:, sl], k1[:], fbck[0:32, sl])
                k3 = rt.tile([32, 512], BF16, name="k1", tag="k1")
                k4 = rt.tile([32, 512], BF16, name="k2", tag="k2")
                nc.vector.tensor_mul(k3[:], kvsb[0:32, sl], nsin4[0:32, sl])
                nc.vector.tensor_mul(k4[:], kb0[:, sl], cos4[0:32, sl])
                nc.vector.tensor_add(k3[:], k3[:], k4[:])
                nc.vector.tensor_mul(kr[1][:, sl], k3[:], fbck[0:32, sl])

                # layout assembly (DMA partition/column moves) for this chunk
                bh = slice(2 * n, 2 * (n + 1))
                for h, (rbase, pcol) in enumerate(((0, 0), (64, 0), (0, 1), (64, 1))):
                    src0 = qr[0][32 * h:32 * h + 32, sl].rearrange(
                        "p (b s) -> p b s", b=2)
                    src1 = qr[1][32 * h:32 * h + 32, sl].rearrange(
                        "p (b s) -> p b s", b=2)
                    nc.sync.dma_start(qeo[rbase:rbase + 32, bh, pcol, :], src0)
                    nc.sync.dma_start(qeo[rbase + 32:rbase + 64, bh, pcol, :], src1)
                nc.sync.dma_start(kdup[0:32, sl], kr[0][:, sl])
                nc.sync.dma_start(kdup[32:64, sl], kr[1][:, sl])
                nc.sync.dma_start(kdup[64:96, sl], kr[0][:, sl])
                nc.sync.dma_start(kdup[96:128, sl], kr[1][:, sl])

            # ---- attention + interleaved output projection for q-block b ----
            def attn_block(b):
                sq = slice(SQB * b, SQB * (b + 1))
                jmax = 2 * b + 1
                yt = pz.tile([65, 1024], F32, name="yt", tag="pz")
                for j in range(jmax + 1):
                    st = ztile()
                    jc = slice(128 * j, 128 * (j + 1))
                    nc.tensor.matmul(st[:, 0:512], kdup[0:64, jc],
                                     qeo[0:64, b, :, :],
                                     start=True, stop=True)
                    nc.tensor.matmul(st[:, 512:1024], kdup[64:128, jc],
                                     qeo[64:128, b, :, :],
                                     start=True, stop=True, skip_group_check=True)
                    pt = pa.tile([128, 1024], BF16, name="pt", tag="pt")
                    nc.scalar.activation(pt[:], st[:], AF.Exp, bias=zb[:, :])
                    if j >= 2 * b:
                        nc.vector.tensor_mul(pt[:], pt[:], mask_s[j - 2 * b])
                    nc.tensor.matmul(yt[:, 0:512], vsb[:, j, :], pt[:, 0:512],
                                     start=(j == 0), stop=(j == jmax))
                    nc.tensor.matmul(yt[:, 512:1024], vsb[:, j, :], pt[:, 512:1024],
                                     start=(j == 0), stop=(j == jmax),
                                     skip_group_check=True)

                # denominators: yt row 64 = sum exp per (head, query)
                # (staged to a base-0 tile: partition_broadcast reads partition 0)
                dnb = pn.tile([1, 1024], F32, name="dnb", tag="dnb")
                nc.vector.tensor_copy(dnb[:], yt[64:65, :])
                rbb = pn.tile([64, 1024], F32, name="rbb", tag="rbb")
                nc.gpsimd.partition_broadcast(rbb[:], dnb[:])
                rbs = pn.tile([64, 1024], F32, name="rbs", tag="rbs")
                nc.vector.reciprocal_approx_fast(rbs[:], rbb[:])
                # normalize: yn0 rows = [h0 | h2], yn1 rows = [h1 | h3]
                # (upper halves staged at base 0 then DMA-moved: walrus requires
                # tensor_tensor dst/src start partitions to match)
                for m in range(2):
                    nc.vector.tensor_mul(yn[m][0:64, sq],
                                         yt[0:64, 512 * m:512 * m + 256],
                                         rbs[:, 512 * m:512 * m + 256])
                    stg = pa.tile([64, 256], BF16, name="stg", tag="stg", bufs=2)
                    nc.vector.tensor_mul(stg[:],
                                         yt[0:64, 512 * m + 256:512 * m + 512],
                                         rbs[:, 512 * m + 256:512 * m + 512])
                    nc.sync.dma_start(yn[m][64:128, sq], stg[:])

                # output projection for the two 128-row s-tiles of this block
                for t in (2 * b, 2 * b + 1):
                    ssl = slice(128 * t, 128 * (t + 1))
                    pot = ztile()
                    for nh in range(2):
                        nsl = slice(512 * nh, 512 * (nh + 1))
                        for kk in range(2):
                            nc.tensor.matmul(
                                pot[:, nsl], yn[kk][:, ssl],
                                wo[:, kk * D + 512 * nh:kk * D + 512 * (nh + 1)],
                                start=(kk == 0), stop=(kk == 1),
                                skip_group_check=(nh == 1))
                    ot = ob.tile([128, D], BF16, name="ot", tag="ot")
                    nc.vector.tensor_copy(ot[:], pot[:])
                    nc.sync.dma_start(out_d[ssl, :], ot[:])

            # ---- emission: all phase-1 chunks, then attention blocks; the
            # shared PSUM pool lets early blocks overlap late chunks without
            # convoying the chunk stream behind attention slot demand ----
            for n in range(NS5):
                p1_chunk(n)
            for b in range(NB):
                attn_block(b)

    nc.finalize()
    return nc


_NC = None


def _get_nc():
    global _NC
    if _NC is None:
        _NC = _build()
    return _NC


def _perm():
    tops = [h * 64 + i for h in range(HG) for i in range(32)]
    bots = [h * 64 + 32 + i for h in range(HG) for i in range(32)]
    return tops + bots


def build_inmaps(x, Wq, Wk, Wv, Wo, q_gain):
    x = np.asarray(x, dtype=np.float32)
    Wq = np.asarray(Wq, dtype=np.float32)
    Wk = np.asarray(Wk, dtype=np.float32)
    Wv = np.asarray(Wv, dtype=np.float32)
    Wo = np.asarray(Wo, dtype=np.float32)
    q_gain = np.asarray(q_gain, dtype=np.float32)

    perm = _perm()
    in_maps = []
    for c in range(8):
        dp, tp = divmod(c, 4)
        # xt[p, k*S+s] = x[dp][s, 128k+p]
        xt_p = np.ascontiguousarray(
            x[dp].reshape(S, NK, 128).transpose(2, 1, 0).reshape(128, NK * S)
        ).astype(BF16NP)
        wq_sel = Wq[tp * E:(tp + 1) * E].T[:, perm]          # [D, 256] permuted
        wq_p = np.ascontiguousarray(
            wq_sel.reshape(NK, 128, E).transpose(1, 0, 2).reshape(128, NK * E)
        ).astype(BF16NP)
        wk_sel = Wk[tp * HD:(tp + 1) * HD].T                  # [D, 64]
        wv_sel = Wv[tp * HD:(tp + 1) * HD].T
        wkv_sel = np.concatenate([wk_sel, wv_sel], axis=1)    # [D, 128]
        wkv_p = np.ascontiguousarray(
            wkv_sel.reshape(NK, 128, 128).transpose(1, 0, 2).reshape(128, NK * 128)
        ).astype(BF16NP)
        # wo rows ordered [h0, h2, h1, h3] to match yn stacking
        horder = [0, 2, 1, 3]
        wo_cols = np.concatenate(
            [np.arange(tp * E + h * HD, tp * E + (h + 1) * HD) for h in horder])
        wo_sel = Wo[:, wo_cols].T                             # [256, D]
        wo_p = np.ascontiguousarray(
            wo_sel.reshape(2, 128, D).transpose(1, 0, 2).reshape(128, 2 * D)
        ).astype(BF16NP)
        g = q_gain[tp * HG:(tp + 1) * HG].astype(np.float64)
        qg8 = (g / 8.0).astype(np.float32).reshape(4, 1)
        in_maps.append({
            "xt": xt_p, "wq": wq_p, "wkv": wkv_p, "wo": wo_p, "qg8": qg8,
        })
    return in_maps


def kernel(x, Wq, Wk, Wv, Wo, q_gain):
    in_maps = build_inmaps(x, Wq, Wk, Wv, Wo, q_gain)
    nc = _get_nc()
    res = run_bass_kernel_spmd(nc, in_maps, core_ids=list(range(8)))
    out = np.zeros((B, S, D), dtype=np.float32)
    for c in range(8):
        out[c // 4] += res.results[c]["out"].astype(np.float32)
    return out


# revision 27
# speedup vs baseline: 1.3172x; 1.2669x over previous
"""Trainium2 Bass kernel for causal GQA self-attention (B=2,S=2048,D=1024,H=16,HKV=4,HD=64).

Sharding: 8 cores = DP(2 over batch) x TP(4 over GQA groups).
Each core computes, for one batch element and one GQA group (4 q heads + 1 kv head),
the partial output  y_group @ Wo[:, group_cols].T  (row-sharded Wo).
Host sums the 4 TP partials per batch element.

v4: single shared PSUM pool (4 slots x 2 banks) across all phases; phase-1
projection/rope streamed per 512-column chunk and attention blocks emitted as
soon as their inputs exist, so the whole kernel is one continuous pipeline.
N=512 attention matmuls (head pairs packed in columns), ScalarE reserved for
softmax exp (+ rms Sqrt), GpSimd does the denominator broadcast.
"""

import sys
from contextlib import ExitStack

sys.path.insert(0, "/opt/trn_rl_repo")

import numpy as np
import ml_dtypes

import concourse.bass as bass
import concourse.bacc as bacc
import concourse.tile as tile
import concourse.mybir as mybir
from concourse.bass_utils import run_bass_kernel_spmd

BF16 = mybir.dt.bfloat16
F32 = mybir.dt.float32
AF = mybir.ActivationFunctionType
BF16NP = ml_dtypes.bfloat16

D, H, HKV, HD, B, S = 1024, 16, 4, 64, 2, 2048
HG = 4              # q heads per core
KV_DIM = HKV * HD   # 256
E = HG * HD         # 256 local q-proj dim
ROPE_BASE = 10000.0
EPS = float(np.finfo(np.float32).eps)

NK = D // 128       # 8 contraction tiles for qkv projections
SQB = 256           # sq block size in attention
NB = S // SQB       # 8 blocks
NJ = S // 128       # 16 sk tiles
NS5 = S // 512      # 4 n-tiles of 512 in projections

# const block column offsets (bf16 [128, CW])
_CO_COS = 0
_CO_SIN = _CO_COS + S
_CO_NSIN = _CO_SIN + S
_CO_M0 = _CO_NSIN + S
_CO_M1 = _CO_M0 + HG * SQB
_CO_ID = _CO_M1 + HG * SQB
_CO_SEL = _CO_ID + 128          # sel4 [128,4]
_CO_BSEL = _CO_SEL + 4          # bsel4 [4,128]
_CO_O64C = _CO_BSEL + 128       # ones64col [64,1]
_CO_O64R = _CO_O64C + 1         # ones64 row [1,64]
CW = _CO_O64R + 64


def _consts():
    """Constant block baked into the NEFF (same for every core): [128, CW] bf16."""
    blk = np.zeros((128, CW), dtype=BF16NP)
    i = np.arange(32, dtype=np.float64)
    inv_freq = 1.0 / (ROPE_BASE ** (2.0 * i / HD))
    pos = np.arange(S, dtype=np.float64)
    fr = pos[:, None] * inv_freq[None, :]          # [S, 32]
    cosT = np.cos(fr).T.astype(np.float32)          # [32, S]
    sinT = np.sin(fr).T.astype(np.float32)
    blk[:, _CO_COS:_CO_COS + S] = np.tile(cosT, (4, 1)).astype(BF16NP)
    blk[:, _CO_SIN:_CO_SIN + S] = np.tile(sinT, (4, 1)).astype(BF16NP)
    blk[:, _CO_NSIN:_CO_NSIN + S] = (-np.tile(sinT, (4, 1))).astype(BF16NP)

    # causal masks for diagonal sk-tiles: pattern p in {0,1}
    # valid iff c >= 128*p + r   (r: sk row 0..127, c: sq col 0..255)
    r = np.arange(128)[:, None]
    c = np.arange(SQB)[None, :]
    for p, co in ((0, _CO_M0), (1, _CO_M1)):
        m = (c >= 128 * p + r).astype(BF16NP)       # [128, 256]
        blk[:, co:co + HG * SQB] = np.tile(m, (1, HG))

    blk[:, _CO_ID:_CO_ID + 128] = np.eye(128, dtype=BF16NP)
    sel4 = np.zeros((128, 4), dtype=BF16NP)         # sumsq selector: tops of head h
    for h in range(4):
        sel4[32 * h:32 * h + 32, h] = 1.0
    blk[:, _CO_SEL:_CO_SEL + 4] = sel4
    bsel4 = np.zeros((4, 128), dtype=BF16NP)        # broadcast f[h] -> rows 32h..32h+32
    for h in range(4):
        bsel4[h, 32 * h:32 * h + 32] = 1.0
    blk[0:4, _CO_BSEL:_CO_BSEL + 128] = bsel4
    blk[0:64, _CO_O64C] = 1.0                       # ones64col [64,1]
    blk[0:1, _CO_O64R:_CO_O64R + 64] = 1.0          # ones64 row [1,64]
    return blk


def _build():
    nc = bacc.Bacc("TRN2", debug=False)

    xt_d = nc.dram_tensor("xt", [128, NK * S], BF16, kind="ExternalInput")
    wq_d = nc.dram_tensor("wq", [128, NK * E], BF16, kind="ExternalInput")
    wkv_d = nc.dram_tensor("wkv", [128, NK * 128], BF16, kind="ExternalInput")
    wo_d = nc.dram_tensor("wo", [128, 2 * D], BF16, kind="ExternalInput")
    qg8_d = nc.dram_tensor("qg8", [4, 1], F32, kind="ExternalInput")
    out_d = nc.dram_tensor("out", [S, D], BF16, kind="ExternalOutput")

    cblk_d = nc.inline_tensor(_consts(), "cblk")

    with tile.TileContext(nc) as tc, ExitStack() as ctx:
        sp = ctx.enter_context(tc.tile_pool(name="static", bufs=1))

        def stile(shape, dt, tag):
            return sp.tile(shape, dt, name=tag, tag=tag)

        # ---- static SBUF tensors ----
        xt = stile([128, NK * S], BF16, "xt")
        wq = stile([128, NK * E], BF16, "wq")
        wkv = stile([128, NK * 128], BF16, "wkv")
        wo = stile([128, 2 * D], BF16, "wo")
        cb = stile([128, CW], BF16, "cb")
        qg8_s = stile([4, 1], F32, "qg8")
        epsb = stile([128, 1], F32, "epsb")
        zb = stile([128, 1], F32, "zb")

        # const views
        cos4 = cb[:, _CO_COS:_CO_COS + S]
        sin4 = cb[:, _CO_SIN:_CO_SIN + S]
        nsin4 = cb[:, _CO_NSIN:_CO_NSIN + S]
        mask_s = [cb[:, _CO_M0:_CO_M0 + HG * SQB], cb[:, _CO_M1:_CO_M1 + HG * SQB]]
        id128 = cb[:, _CO_ID:_CO_ID + 128]
        sel4 = cb[:, _CO_SEL:_CO_SEL + 4]
        bsel4 = cb[0:4, _CO_BSEL:_CO_BSEL + 128]
        ones64col = cb[0:64, _CO_O64C:_CO_O64C + 1]
        ones64row = cb[0:1, _CO_O64R:_CO_O64R + 64]

        qsb = [stile([128, S], BF16, f"qsb{m}") for m in range(2)]   # T/B packed
        kvsb = stile([128, S], BF16, "kvsb")                          # k(0:64) | v(64:128)
        sqq = [stile([128, S], BF16, f"sqq{m}") for m in range(2)]
        sqkv = stile([64, S], BF16, "sqkv")
        fq = stile([4, S], BF16, "fq")
        fk = stile([1, S], BF16, "fk")
        fbcq = stile([128, S], BF16, "fbcq")
        fbck = stile([64, S], BF16, "fbck")
        qr = [stile([128, S], BF16, f"qr{m}") for m in range(2)]      # rotated T/B
        kr = [stile([32, S], BF16, f"kr{m}") for m in range(2)]
        kb0 = stile([32, S], BF16, "kb0")
        qeo = stile([128, NB, 2, SQB], BF16, "qeo")   # [he|ho] x per-b [pair0|pair1]
        kdup = stile([128, S], BF16, "kdup")
        vsb = stile([128, NJ, 65], BF16, "vsb")       # [v | ones]
        yn = [stile([128, S], BF16, f"yn{m}") for m in range(2)]      # normalized y^T

        # ---- load everything (xt on the sync HWDGE ring; consts/weights on the
        # scalar ring so the two streams transfer concurrently) ----
        nc.sync.dma_start(wq[:], wq_d[:])
        nc.sync.dma_start(wkv[:], wkv_d[:])
        nc.sync.dma_start(qg8_s[:], qg8_d[:])
        for kc in range(4):
            lsl = slice(kc * 2 * S, (kc + 1) * 2 * S)
            nc.sync.dma_start(xt[:, lsl], xt_d[:, lsl])
        nc.scalar.dma_start(cb[:], cblk_d[:])
        nc.scalar.dma_start(wo[:], wo_d[:])
        nc.vector.memset(vsb[:], 1.0)  # ones column at [:, j, 64]; 0:64 overwritten below
        nc.vector.memset(epsb[:], EPS)
        nc.vector.memset(zb[:], 0.0)

        with (
            tc.tile_pool(name="lns", bufs=2) as lns,
            tc.tile_pool(name="rt", bufs=2) as rt,
            tc.tile_pool(name="pa", bufs=4) as pa,
            tc.tile_pool(name="pn", bufs=1) as pn,
            tc.tile_pool(name="ob", bufs=2) as ob,
            ExitStack() as pctx,
        ):
            pz = pctx.enter_context(
                tc.tile_pool(name="pz", bufs=4, space=bass.MemorySpace.PSUM))

            def ztile(shape=(128, 1024), dt=F32):
                return pz.tile(list(shape), dt, name="pz", tag="pz")

            # ---- phase 1 chunk: projections + rms factors + rope + assembly
            # for s-columns 512n..512n+512 ----
            def p1_chunk(n):
                sl = slice(512 * n, 512 * (n + 1))
                pq01 = ztile()                 # m0: cols 0:512, m1: cols 512:1024
                pk8 = ztile()                  # pkv: cols 0:512; v-transposes in bank B
                for k in range(NK):
                    xsl = xt[:, k * S + 512 * n:k * S + 512 * (n + 1)]
                    st_, sp_ = (k == 0), (k == NK - 1)
                    nc.tensor.matmul(pq01[:, 0:512], wq[:, k * E:k * E + 128],
                                     xsl, start=st_, stop=sp_)
                    nc.tensor.matmul(pq01[:, 512:1024],
                                     wq[:, k * E + 128:k * E + 256],
                                     xsl, start=st_, stop=sp_, skip_group_check=True)
                    nc.tensor.matmul(pk8[:, 0:512], wkv[:, k * 128:(k + 1) * 128],
                                     xsl, start=st_, stop=sp_)
                for m in range(2):
                    nc.vector.tensor_copy(qsb[m][:, sl], pq01[:, 512 * m:512 * (m + 1)])
                    nc.vector.tensor_mul(sqq[m][:, sl], qsb[m][:, sl], qsb[m][:, sl])
                nc.vector.tensor_copy(kvsb[:, sl], pk8[:, 0:512])
                nc.vector.tensor_mul(sqkv[:, sl], kvsb[0:64, sl], kvsb[0:64, sl])
                nc.sync.dma_start(kb0[:, sl], kvsb[32:64, sl])
                # v transpose: [64,128] slices -> [128,64] (into pk8 bank B)
                for t in range(4):
                    st_ = 4 * n + t
                    ptr = pk8[:, 512 + 32 * t:512 + 32 * (t + 1)].bitcast(BF16)
                    nc.tensor.transpose(
                        ptr, kvsb[64:128, 128 * st_:128 * (st_ + 1)],
                        id128[64:128, 64:128])
                    nc.vector.tensor_copy(vsb[:, st_, 0:64], ptr)

                # rms factors: f = gain/8 * (ssq/HD + eps)^-1/2 (Sqrt + fast recip)
                pf = ztile((33, 1024))         # psq rows 0:4 bank A; psk row 32 bank B
                psq = pf[0:4, 0:512]
                psk = pf[32:33, 512:1024]
                nc.tensor.matmul(psq, sel4, sqq[0][:, sl], start=True, stop=False)
                nc.tensor.matmul(psq, sel4, sqq[1][:, sl], start=False, stop=True)
                nc.tensor.matmul(psk, ones64col, sqkv[:, sl], start=True, stop=True,
                                 skip_group_check=True)
                fsq = lns.tile([4, 512], F32, name="fsq", tag="fsq")
                nc.scalar.activation(fsq[:], psq, AF.Sqrt, scale=1.0 / HD,
                                     bias=epsb[0:4, :])
                frq = lns.tile([4, 512], F32, name="frq", tag="frq")
                nc.vector.reciprocal_approx_fast(frq[:], fsq[:])
                nc.vector.tensor_scalar_mul(fq[:, sl], frq[:], qg8_s[:, :])
                fsk = lns.tile([1, 512], F32, name="fsk", tag="fsk")
                nc.scalar.activation(fsk[:], psk, AF.Sqrt, scale=1.0 / HD,
                                     bias=epsb[0:1, :])
                frk = lns.tile([1, 512], F32, name="frk", tag="frk")
                nc.vector.reciprocal_approx_fast(frk[:], fsk[:])
                nc.vector.tensor_scalar_mul(fk[:, sl], frk[:], 1.0)
                # broadcast factors along hd rows via PE
                pbx = ztile()                  # pb cols 0:512; pbk cols 512:1024
                nc.tensor.matmul(pbx[:, 0:512], bsel4, fq[:, sl],
                                 start=True, stop=True)
                nc.vector.tensor_copy(fbcq[:, sl], pbx[:, 0:512])
                nc.tensor.matmul(pbx[0:64, 512:1024], ones64row, fk[:, sl],
                                 start=True, stop=True, skip_group_check=True)
                nc.vector.tensor_copy(fbck[:, sl], pbx[0:64, 512:1024])

                # rope + scale (DVE, bf16)
                t1 = rt.tile([128, 512], BF16, name="t1", tag="t1")
                t2 = rt.tile([128, 512], BF16, name="t2", tag="t2")
                nc.vector.tensor_mul(t1[:], qsb[0][:, sl], cos4[:, sl])
                nc.vector.tensor_mul(t2[:], qsb[1][:, sl], sin4[:, sl])
                nc.vector.tensor_add(t1[:], t1[:], t2[:])
                nc.vector.tensor_mul(qr[0][:, sl], t1[:], fbcq[:, sl])
                u1 = rt.tile([128, 512], BF16, name="t1", tag="t1")
                u2 = rt.tile([128, 512], BF16, name="t2", tag="t2")
                nc.vector.tensor_mul(u1[:], qsb[0][:, sl], nsin4[:, sl])
                nc.vector.tensor_mul(u2[:], qsb[1][:, sl], cos4[:, sl])
                nc.vector.tensor_add(u1[:], u1[:], u2[:])
                nc.vector.tensor_mul(qr[1][:, sl], u1[:], fbcq[:, sl])
                k1 = rt.tile([32, 512], BF16, name="k1", tag="k1")
                k2 = rt.tile([32, 512], BF16, name="k2", tag="k2")
                nc.vector.tensor_mul(k1[:], kvsb[0:32, sl], cos4[0:32, sl])
                nc.vector.tensor_mul(k2[:], kb0[:, sl], sin4[0:32, sl])
                nc.vector.tensor_add(k1[:], k1[:], k2[:])
                nc.vector.tensor_mul(kr[0][:, sl], k1[:], fbck[0:32, sl])
                k3 = rt.tile([32, 512], BF16, name="k1", tag="k1")
                k4 = rt.tile([32, 512], BF16, name="k2", tag="k2")
                nc.vector.tensor_mul(k3[:], kvsb[0:32, sl], nsin4[0:32, sl])
                nc.vector.tensor_mul(k4[:], kb0[:, sl], cos4[0:32, sl])
                nc.vector.tensor_add(k3[:], k3[:], k4[:])
                nc.vector.tensor_mul(kr[1][:, sl], k3[:], fbck[0:32, sl])

                # layout assembly (DMA partition/column moves) for this chunk
                bh = slice(2 * n, 2 * (n + 1))
                for h, (rbase, pcol) in enumerate(((0, 0), (64, 0), (0, 1), (64, 1))):
                    src0 = qr[0][32 * h:32 * h + 32, sl].rearrange(
                        "p (b s) -> p b s", b=2)
                    src1 = qr[1][32 * h:32 * h + 32, sl].rearrange(
                        "p (b s) -> p b s", b=2)
                    nc.sync.dma_start(qeo[rbase:rbase + 32, bh, pcol, :], src0)
                    nc.sync.dma_start(qeo[rbase + 32:rbase + 64, bh, pcol, :], src1)
                nc.sync.dma_start(kdup[0:32, sl], kr[0][:, sl])
                nc.sync.dma_start(kdup[32:64, sl], kr[1][:, sl])
                nc.sync.dma_start(kdup[64:96, sl], kr[0][:, sl])
                nc.sync.dma_start(kdup[96:128, sl], kr[1][:, sl])

            # ---- attention + interleaved output projection for q-block b ----
            def attn_block(b):
                sq = slice(SQB * b, SQB * (b + 1))
                jmax = 2 * b + 1
                yt = apools["py"].tile([65, 1024], F32, name="yt", tag="yt")
                for j in range(jmax + 1):
                    st = apools["ps"].tile([128, 1024], F32, name="st", tag="st")
                    jc = slice(128 * j, 128 * (j + 1))
                    nc.tensor.matmul(st[:, 0:512], kdup[0:64, jc],
                                     qeo[0:64, b, :, :],
                                     start=True, stop=True)
                    nc.tensor.matmul(st[:, 512:1024], kdup[64:128, jc],
                                     qeo[64:128, b, :, :],
                                     start=True, stop=True, skip_group_check=True)
                    pt = pa.tile([128, 1024], BF16, name="pt", tag="pt")
                    nc.scalar.activation(pt[:], st[:], AF.Exp, bias=zb[:, :])
                    if j >= 2 * b:
                        nc.vector.tensor_mul(pt[:], pt[:], mask_s[j - 2 * b])
                    nc.tensor.matmul(yt[:, 0:512], vsb[:, j, :], pt[:, 0:512],
                                     start=(j == 0), stop=(j == jmax))
                    nc.tensor.matmul(yt[:, 512:1024], vsb[:, j, :], pt[:, 512:1024],
                                     start=(j == 0), stop=(j == jmax),
                                     skip_group_check=True)

                # evacuate yt to SBUF promptly so the single yt slot frees for b+1
                ytc = pn.tile([65, 1024], F32, name="ytc", tag="ytc")
                nc.vector.tensor_copy(ytc[:], yt[:])
                # denominators: ytc row 64 = sum exp per (head, query)
                # (staged to a base-0 tile: partition_broadcast reads partition 0)
                dnb = pn.tile([1, 1024], F32, name="dnb", tag="dnb")
                nc.vector.tensor_copy(dnb[:], ytc[64:65, :])
                rbb = pn.tile([64, 1024], F32, name="rbb", tag="rbb")
                nc.gpsimd.partition_broadcast(rbb[:], dnb[:])
                rbs = pn.tile([64, 1024], F32, name="rbs", tag="rbs")
                nc.vector.reciprocal_approx_fast(rbs[:], rbb[:])
                # normalize: yn0 rows = [h0 | h2], yn1 rows = [h1 | h3]
                # (upper halves staged at base 0 then DMA-moved: walrus requires
                # tensor_tensor dst/src start partitions to match)
                for m in range(2):
                    nc.vector.tensor_mul(yn[m][0:64, sq],
                                         ytc[0:64, 512 * m:512 * m + 256],
                                         rbs[:, 512 * m:512 * m + 256])
                    stg = pa.tile([64, 256], BF16, name="stg", tag="stg", bufs=2)
                    nc.vector.tensor_mul(stg[:],
                                         ytc[0:64, 512 * m + 256:512 * m + 512],
                                         rbs[:, 512 * m + 256:512 * m + 512])
                    nc.sync.dma_start(yn[m][64:128, sq], stg[:])

                # output projection for the two 128-row s-tiles of this block
                for t in (2 * b, 2 * b + 1):
                    ssl = slice(128 * t, 128 * (t + 1))
                    pot = apools["po"].tile([128, 1024], F32, name="pot", tag="pot")
                    for nh in range(2):
                        nsl = slice(512 * nh, 512 * (nh + 1))
                        for kk in range(2):
                            nc.tensor.matmul(
                                pot[:, nsl], yn[kk][:, ssl],
                                wo[:, kk * D + 512 * nh:kk * D + 512 * (nh + 1)],
                                start=(kk == 0), stop=(kk == 1),
                                skip_group_check=(nh == 1))
                    ot = ob.tile([128, D], BF16, name="ot", tag="ot")
                    nc.vector.tensor_copy(ot[:], pot[:])
                    nc.sync.dma_start(out_d[ssl, :], ot[:])

            # ---- emission: all phase-1 chunks (own PSUM pool), then the
            # attention blocks with their own PSUM pools ----
            apools = {}
            for n in range(NS5):
                p1_chunk(n)
            pctx.close()
            apools["ps"] = pctx.enter_context(
                tc.tile_pool(name="ps", bufs=2, space=bass.MemorySpace.PSUM))
            apools["py"] = pctx.enter_context(
                tc.tile_pool(name="py", bufs=1, space=bass.MemorySpace.PSUM))
            apools["po"] = pctx.enter_context(
                tc.tile_pool(name="po", bufs=1, space=bass.MemorySpace.PSUM))
            for b in range(NB):
                attn_block(b)

    nc.finalize()
    return nc


_NC = None


def _get_nc():
    global _NC
    if _NC is None:
        _NC = _build()
    return _NC


def _perm():
    tops = [h * 64 + i for h in range(HG) for i in range(32)]
    bots = [h * 64 + 32 + i for h in range(HG) for i in range(32)]
    return tops + bots


def build_inmaps(x, Wq, Wk, Wv, Wo, q_gain):
    x = np.asarray(x, dtype=np.float32)
    Wq = np.asarray(Wq, dtype=np.float32)
    Wk = np.asarray(Wk, dtype=np.float32)
    Wv = np.asarray(Wv, dtype=np.float32)
    Wo = np.asarray(Wo, dtype=np.float32)
    q_gain = np.asarray(q_gain, dtype=np.float32)

    perm = _perm()
    in_maps = []
    for c in range(8):
        dp, tp = divmod(c, 4)
        # xt[p, k*S+s] = x[dp][s, 128k+p]
        xt_p = np.ascontiguousarray(
            x[dp].reshape(S, NK, 128).transpose(2, 1, 0).reshape(128, NK * S)
        ).astype(BF16NP)
        wq_sel = Wq[tp * E:(tp + 1) * E].T[:, perm]          # [D, 256] permuted
        wq_p = np.ascontiguousarray(
            wq_sel.reshape(NK, 128, E).transpose(1, 0, 2).reshape(128, NK * E)
        ).astype(BF16NP)
        wk_sel = Wk[tp * HD:(tp + 1) * HD].T                  # [D, 64]
        wv_sel = Wv[tp * HD:(tp + 1) * HD].T
        wkv_sel = np.concatenate([wk_sel, wv_sel], axis=1)    # [D, 128]
        wkv_p = np.ascontiguousarray(
            wkv_sel.reshape(NK, 128, 128).transpose(1, 0, 2).reshape(128, NK * 128)
        ).astype(BF16NP)
        # wo rows ordered [h0, h2, h1, h3] to match yn stacking
        horder = [0, 2, 1, 3]
        wo_cols = np.concatenate(
            [np.arange(tp * E + h * HD, tp * E + (h + 1) * HD) for h in horder])
        wo_sel = Wo[:, wo_cols].T                             # [256, D]
        wo_p = np.ascontiguousarray(
            wo_sel.reshape(2, 128, D).transpose(1, 0, 2).reshape(128, 2 * D)
        ).astype(BF16NP)
        g = q_gain[tp * HG:(tp + 1) * HG].astype(np.float64)
        qg8 = (g / 8.0).astype(np.float32).reshape(4, 1)
        in_maps.append({
            "xt": xt_p, "wq": wq_p, "wkv": wkv_p, "wo": wo_p, "qg8": qg8,
        })
    return in_maps


def kernel(x, Wq, Wk, Wv, Wo, q_gain):
    in_maps = build_inmaps(x, Wq, Wk, Wv, Wo, q_gain)
    nc = _get_nc()
    res = run_bass_kernel_spmd(nc, in_maps, core_ids=list(range(8)))
    out = np.zeros((B, S, D), dtype=np.float32)
    for c in range(8):
        out[c // 4] += res.results[c]["out"].astype(np.float32)
    return out


# revision 29
# speedup vs baseline: 1.6722x; 1.2695x over previous
"""Trainium2 Bass kernel for causal GQA self-attention (B=2,S=2048,D=1024,H=16,HKV=4,HD=64).

Sharding: 8 cores = DP(2 over batch) x TP(4 over GQA groups).
Each core computes, for one batch element and one GQA group (4 q heads + 1 kv head),
the partial output  y_group @ Wo[:, group_cols].T  (row-sharded Wo).
Host sums the 4 TP partials per batch element.

v4: single shared PSUM pool (4 slots x 2 banks) across all phases; phase-1
projection/rope streamed per 512-column chunk and attention blocks emitted as
soon as their inputs exist, so the whole kernel is one continuous pipeline.
N=512 attention matmuls (head pairs packed in columns), ScalarE reserved for
softmax exp (+ rms Sqrt), GpSimd does the denominator broadcast.
"""

import sys
from contextlib import ExitStack

sys.path.insert(0, "/opt/trn_rl_repo")

import numpy as np
import ml_dtypes

import concourse.bass as bass
import concourse.bacc as bacc
import concourse.tile as tile
import concourse.mybir as mybir
from concourse.bass_utils import run_bass_kernel_spmd

BF16 = mybir.dt.bfloat16
F32 = mybir.dt.float32
AF = mybir.ActivationFunctionType
BF16NP = ml_dtypes.bfloat16

D, H, HKV, HD, B, S = 1024, 16, 4, 64, 2, 2048
HG = 4              # q heads per core
KV_DIM = HKV * HD   # 256
E = HG * HD         # 256 local q-proj dim
ROPE_BASE = 10000.0
EPS = float(np.finfo(np.float32).eps)

NK = D // 128       # 8 contraction tiles for qkv projections
SQB = 256           # sq block size in attention
NB = S // SQB       # 8 blocks
NJ = S // 128       # 16 sk tiles
NS5 = S // 512      # 4 n-tiles of 512 in projections

# const block column offsets (bf16 [128, CW])
_CO_COS = 0
_CO_SIN = _CO_COS + S
_CO_NSIN = _CO_SIN + S
_CO_M0 = _CO_NSIN + S
_CO_M1 = _CO_M0 + HG * SQB
_CO_ID = _CO_M1 + HG * SQB
_CO_SEL = _CO_ID + 128          # sel4 [128,4]
_CO_BSEL = _CO_SEL + 4          # bsel4 [4,128]
_CO_O64C = _CO_BSEL + 128       # ones64col [64,1]
_CO_O64R = _CO_O64C + 1         # ones64 row [1,64]
CW = _CO_O64R + 64


def _consts():
    """Constant block baked into the NEFF (same for every core): [128, CW] bf16."""
    blk = np.zeros((128, CW), dtype=BF16NP)
    i = np.arange(32, dtype=np.float64)
    inv_freq = 1.0 / (ROPE_BASE ** (2.0 * i / HD))
    pos = np.arange(S, dtype=np.float64)
    fr = pos[:, None] * inv_freq[None, :]          # [S, 32]
    cosT = np.cos(fr).T.astype(np.float32)          # [32, S]
    sinT = np.sin(fr).T.astype(np.float32)
    blk[:, _CO_COS:_CO_COS + S] = np.tile(cosT, (4, 1)).astype(BF16NP)
    blk[:, _CO_SIN:_CO_SIN + S] = np.tile(sinT, (4, 1)).astype(BF16NP)
    blk[:, _CO_NSIN:_CO_NSIN + S] = (-np.tile(sinT, (4, 1))).astype(BF16NP)

    # causal masks for diagonal sk-tiles: pattern p in {0,1}
    # valid iff c >= 128*p + r   (r: sk row 0..127, c: sq col 0..255)
    r = np.arange(128)[:, None]
    c = np.arange(SQB)[None, :]
    for p, co in ((0, _CO_M0), (1, _CO_M1)):
        m = (c >= 128 * p + r).astype(BF16NP)       # [128, 256]
        blk[:, co:co + HG * SQB] = np.tile(m, (1, HG))

    blk[:, _CO_ID:_CO_ID + 128] = np.eye(128, dtype=BF16NP)
    sel4 = np.zeros((128, 4), dtype=BF16NP)         # sumsq selector: tops of head h
    for h in range(4):
        sel4[32 * h:32 * h + 32, h] = 1.0
    blk[:, _CO_SEL:_CO_SEL + 4] = sel4
    bsel4 = np.zeros((4, 128), dtype=BF16NP)        # broadcast f[h] -> rows 32h..32h+32
    for h in range(4):
        bsel4[h, 32 * h:32 * h + 32] = 1.0
    blk[0:4, _CO_BSEL:_CO_BSEL + 128] = bsel4
    blk[0:64, _CO_O64C] = 1.0                       # ones64col [64,1]
    blk[0:1, _CO_O64R:_CO_O64R + 64] = 1.0          # ones64 row [1,64]
    return blk


def _build():
    nc = bacc.Bacc("TRN2", debug=False)

    xt_d = nc.dram_tensor("xt", [128, NK * S], BF16, kind="ExternalInput")
    wq_d = nc.dram_tensor("wq", [128, NK * E], BF16, kind="ExternalInput")
    wkv_d = nc.dram_tensor("wkv", [128, NK * 128], BF16, kind="ExternalInput")
    wo_d = nc.dram_tensor("wo", [128, 2 * D], BF16, kind="ExternalInput")
    qg8_d = nc.dram_tensor("qg8", [4, 1], F32, kind="ExternalInput")
    out_d = nc.dram_tensor("out", [S, D], BF16, kind="ExternalOutput")

    cblk_d = nc.inline_tensor(_consts(), "cblk")

    with tile.TileContext(nc) as tc, ExitStack() as ctx:
        sp = ctx.enter_context(tc.tile_pool(name="static", bufs=1))

        def stile(shape, dt, tag):
            return sp.tile(shape, dt, name=tag, tag=tag)

        # ---- static SBUF tensors ----
        xt = stile([128, NK * S], BF16, "xt")
        wq = stile([128, NK * E], BF16, "wq")
        wkv = stile([128, NK * 128], BF16, "wkv")
        wo = stile([128, 2 * D], BF16, "wo")
        cb = stile([128, CW], BF16, "cb")
        qg8_s = stile([4, 1], F32, "qg8")
        epsb = stile([128, 1], F32, "epsb")
        zb = stile([128, 1], F32, "zb")

        # const views
        cos4 = cb[:, _CO_COS:_CO_COS + S]
        sin4 = cb[:, _CO_SIN:_CO_SIN + S]
        nsin4 = cb[:, _CO_NSIN:_CO_NSIN + S]
        mask_s = [cb[:, _CO_M0:_CO_M0 + HG * SQB], cb[:, _CO_M1:_CO_M1 + HG * SQB]]
        id128 = cb[:, _CO_ID:_CO_ID + 128]
        sel4 = cb[:, _CO_SEL:_CO_SEL + 4]
        bsel4 = cb[0:4, _CO_BSEL:_CO_BSEL + 128]
        ones64col = cb[0:64, _CO_O64C:_CO_O64C + 1]
        ones64row = cb[0:1, _CO_O64R:_CO_O64R + 64]

        qsb = [stile([128, S], BF16, f"qsb{m}") for m in range(2)]   # T/B packed
        kvsb = stile([128, S], BF16, "kvsb")                          # k(0:64) | v(64:128)
        sqq = [stile([128, S], BF16, f"sqq{m}") for m in range(2)]
        sqkv = stile([64, S], BF16, "sqkv")
        fq = stile([4, S], BF16, "fq")
        fk = stile([1, S], BF16, "fk")
        fbcq = stile([128, S], BF16, "fbcq")
        fbck = stile([64, S], BF16, "fbck")
        qr = [stile([128, S], BF16, f"qr{m}") for m in range(2)]      # rotated T/B
        kr = [stile([32, S], BF16, f"kr{m}") for m in range(2)]
        kb0 = stile([32, S], BF16, "kb0")
        qeo = stile([128, NB, 2, SQB], BF16, "qeo")   # [he|ho] x per-b [pair0|pair1]
        kdup = stile([128, S], BF16, "kdup")
        vsb = stile([128, NJ, 65], BF16, "vsb")       # [v | ones]
        yn = [stile([128, S], BF16, f"yn{m}") for m in range(2)]      # normalized y^T

        # ---- load everything (xt on the sync HWDGE ring; consts/weights on the
        # scalar ring so the two streams transfer concurrently) ----
        nc.sync.dma_start(wq[:], wq_d[:])
        nc.sync.dma_start(wkv[:], wkv_d[:])
        nc.sync.dma_start(qg8_s[:], qg8_d[:])
        for kc in range(4):
            lsl = slice(kc * 2 * S, (kc + 1) * 2 * S)
            nc.sync.dma_start(xt[:, lsl], xt_d[:, lsl])
        nc.scalar.dma_start(cb[:], cblk_d[:])
        nc.scalar.dma_start(wo[:], wo_d[:])
        nc.vector.memset(vsb[:], 1.0)  # ones column at [:, j, 64]; 0:64 overwritten below
        nc.vector.memset(epsb[:], EPS)
        nc.vector.memset(zb[:], 0.0)

        with (
            tc.tile_pool(name="lns", bufs=1) as lns,
            tc.tile_pool(name="rt", bufs=2) as rt,
            tc.tile_pool(name="pa", bufs=4) as pa,
            tc.tile_pool(name="pn", bufs=1) as pn,
            tc.tile_pool(name="ob", bufs=2) as ob,
            ExitStack() as pctx,
        ):
            pz = pctx.enter_context(
                tc.tile_pool(name="pz", bufs=4, space=bass.MemorySpace.PSUM))

            def ztile(shape=(128, 1024), dt=F32):
                return pz.tile(list(shape), dt, name="pz", tag="pz")

            # ---- phase 1: projections per 512-col chunk; factors + rope +
            # assembly per 1024-col block (fewer, larger DVE/ACT ops) ----
            def p1_proj(n):
                sl = slice(512 * n, 512 * (n + 1))
                pq01 = ztile()                 # m0: cols 0:512, m1: cols 512:1024
                pk8 = ztile()                  # pkv: cols 0:512; v-transposes in bank B
                for k in range(NK):
                    xsl = xt[:, k * S + 512 * n:k * S + 512 * (n + 1)]
                    st_, sp_ = (k == 0), (k == NK - 1)
                    nc.tensor.matmul(pq01[:, 0:512], wq[:, k * E:k * E + 128],
                                     xsl, start=st_, stop=sp_)
                    nc.tensor.matmul(pq01[:, 512:1024],
                                     wq[:, k * E + 128:k * E + 256],
                                     xsl, start=st_, stop=sp_, skip_group_check=True)
                    nc.tensor.matmul(pk8[:, 0:512], wkv[:, k * 128:(k + 1) * 128],
                                     xsl, start=st_, stop=sp_)
                for m in range(2):
                    nc.scalar.copy(qsb[m][:, sl], pq01[:, 512 * m:512 * (m + 1)])
                    nc.vector.tensor_mul(sqq[m][:, sl], qsb[m][:, sl], qsb[m][:, sl])
                nc.scalar.copy(kvsb[:, sl], pk8[:, 0:512])
                nc.vector.tensor_mul(sqkv[:, sl], kvsb[0:64, sl], kvsb[0:64, sl])
                nc.sync.dma_start(kb0[:, sl], kvsb[32:64, sl])
                # v transpose: [64,128] slices -> [128,64] (into pk8 bank B)
                for t in range(4):
                    st_ = 4 * n + t
                    ptr = pk8[:, 512 + 32 * t:512 + 32 * (t + 1)].bitcast(BF16)
                    nc.tensor.transpose(
                        ptr, kvsb[64:128, 128 * st_:128 * (st_ + 1)],
                        id128[64:128, 64:128])
                    nc.vector.tensor_copy(vsb[:, st_, 0:64], ptr)

            def p1_tail(g):
                # factors + rope + layout assembly for s-columns 1024g..1024g+1024
                sl = slice(1024 * g, 1024 * (g + 1))
                hs = [slice(1024 * g + 512 * i, 1024 * g + 512 * (i + 1))
                      for i in range(2)]
                # rms factors: f = gain/8 * (ssq/HD + eps)^-1/2
                pf = ztile((128, 1024))   # psq rows 0:4, psk row 32, pbk rows 64:128
                for i in range(2):
                    psq = pf[0:4, 512 * i:512 * (i + 1)]
                    nc.tensor.matmul(psq, sel4, sqq[0][:, hs[i]],
                                     start=True, stop=False, skip_group_check=(i > 0))
                    nc.tensor.matmul(psq, sel4, sqq[1][:, hs[i]],
                                     start=False, stop=True, skip_group_check=True)
                    nc.tensor.matmul(pf[32:33, 512 * i:512 * (i + 1)], ones64col,
                                     sqkv[:, hs[i]], start=True, stop=True,
                                     skip_group_check=True)
                fsq = lns.tile([4, 1024], F32, name="fsq", tag="fsq")
                nc.scalar.activation(fsq[:], pf[0:4, :], AF.Sqrt, scale=1.0 / HD,
                                     bias=epsb[0:4, :])
                frq = lns.tile([4, 1024], F32, name="frq", tag="frq")
                nc.vector.reciprocal_approx_fast(frq[:], fsq[:])
                nc.vector.tensor_scalar_mul(fq[:, sl], frq[:], qg8_s[:, :])
                fsk = lns.tile([1, 1024], F32, name="fsk", tag="fsk")
                nc.scalar.activation(fsk[:], pf[32:33, :], AF.Sqrt, scale=1.0 / HD,
                                     bias=epsb[0:1, :])
                frk = lns.tile([1, 1024], F32, name="frk", tag="frk")
                nc.vector.reciprocal_approx_fast(frk[:], fsk[:])
                nc.vector.tensor_scalar_mul(fk[:, sl], frk[:], 1.0)
                # broadcast factors along hd rows via PE
                pbx = ztile()
                for i in range(2):
                    nc.tensor.matmul(pbx[:, 512 * i:512 * (i + 1)], bsel4,
                                     fq[:, hs[i]], start=True, stop=True,
                                     skip_group_check=(i > 0))
                    nc.tensor.matmul(pf[64:128, 512 * i:512 * (i + 1)], ones64row,
                                     fk[:, hs[i]], start=True, stop=True,
                                     skip_group_check=True)
                nc.scalar.copy(fbcq[:, sl], pbx[:])
                nc.scalar.copy(fbck[:, sl], pf[64:128, :])

                # rope + scale (DVE, bf16)
                t1 = rt.tile([128, 1024], BF16, name="t1", tag="t1")
                t2 = rt.tile([128, 1024], BF16, name="t2", tag="t2")
                nc.vector.tensor_mul(t1[:], qsb[0][:, sl], cos4[:, sl])
                nc.vector.tensor_mul(t2[:], qsb[1][:, sl], sin4[:, sl])
                nc.vector.tensor_add(t1[:], t1[:], t2[:])
                nc.vector.tensor_mul(qr[0][:, sl], t1[:], fbcq[:, sl])
                u1 = rt.tile([128, 1024], BF16, name="t1", tag="t1")
                u2 = rt.tile([128, 1024], BF16, name="t2", tag="t2")
                nc.vector.tensor_mul(u1[:], qsb[0][:, sl], nsin4[:, sl])
                nc.vector.tensor_mul(u2[:], qsb[1][:, sl], cos4[:, sl])
                nc.vector.tensor_add(u1[:], u1[:], u2[:])
                nc.vector.tensor_mul(qr[1][:, sl], u1[:], fbcq[:, sl])
                k1 = rt.tile([32, 1024], BF16, name="k1", tag="k1")
                k2 = rt.tile([32, 1024], BF16, name="k2", tag="k2")
                nc.vector.tensor_mul(k1[:], kvsb[0:32, sl], cos4[0:32, sl])
                nc.vector.tensor_mul(k2[:], kb0[:, sl], sin4[0:32, sl])
                nc.vector.tensor_add(k1[:], k1[:], k2[:])
                nc.vector.tensor_mul(kr[0][:, sl], k1[:], fbck[0:32, sl])
                k3 = rt.tile([32, 1024], BF16, name="k1", tag="k1")
                k4 = rt.tile([32, 1024], BF16, name="k2", tag="k2")
                nc.vector.tensor_mul(k3[:], kvsb[0:32, sl], nsin4[0:32, sl])
                nc.vector.tensor_mul(k4[:], kb0[:, sl], cos4[0:32, sl])
                nc.vector.tensor_add(k3[:], k3[:], k4[:])
                nc.vector.tensor_mul(kr[1][:, sl], k3[:], fbck[0:32, sl])

                # layout assembly (DMA partition/column moves)
                bh = slice(4 * g, 4 * (g + 1))
                for h, (rbase, pcol) in enumerate(((0, 0), (64, 0), (0, 1), (64, 1))):
                    src0 = qr[0][32 * h:32 * h + 32, sl].rearrange(
                        "p (b s) -> p b s", b=4)
                    src1 = qr[1][32 * h:32 * h + 32, sl].rearrange(
                        "p (b s) -> p b s", b=4)
                    nc.sync.dma_start(qeo[rbase:rbase + 32, bh, pcol, :], src0)
                    nc.sync.dma_start(qeo[rbase + 32:rbase + 64, bh, pcol, :], src1)
                nc.sync.dma_start(kdup[0:32, sl], kr[0][:, sl])
                nc.sync.dma_start(kdup[32:64, sl], kr[1][:, sl])
                nc.sync.dma_start(kdup[64:96, sl], kr[0][:, sl])
                nc.sync.dma_start(kdup[96:128, sl], kr[1][:, sl])

            # ---- attention + interleaved output projection for q-block b ----
            def attn_block(b):
                sq = slice(SQB * b, SQB * (b + 1))
                jmax = 2 * b + 1
                yt = apools["py"].tile([65, 1024], F32, name="yt", tag="yt")
                for j in range(jmax + 1):
                    st = apools["ps"].tile([128, 1024], F32, name="st", tag="st")
                    jc = slice(128 * j, 128 * (j + 1))
                    nc.tensor.matmul(st[:, 0:512], kdup[0:64, jc],
                                     qeo[0:64, b, :, :],
                                     start=True, stop=True)
                    nc.tensor.matmul(st[:, 512:1024], kdup[64:128, jc],
                                     qeo[64:128, b, :, :],
                                     start=True, stop=True, skip_group_check=True)
                    pt = pa.tile([128, 1024], BF16, name="pt", tag="pt")
                    nc.scalar.activation(pt[:], st[:], AF.Exp, bias=zb[:, :])
                    if j >= 2 * b:
                        nc.vector.tensor_mul(pt[:], pt[:], mask_s[j - 2 * b])
                    nc.tensor.matmul(yt[:, 0:512], vsb[:, j, :], pt[:, 0:512],
                                     start=(j == 0), stop=(j == jmax))
                    nc.tensor.matmul(yt[:, 512:1024], vsb[:, j, :], pt[:, 512:1024],
                                     start=(j == 0), stop=(j == jmax),
                                     skip_group_check=True)

                # evacuate yt to SBUF promptly so the single yt slot frees for b+1
                ytc = pn.tile([65, 1024], F32, name="ytc", tag="ytc")
                nc.vector.tensor_copy(ytc[:], yt[:])
                # denominators: ytc row 64 = sum exp per (head, query)
                # (staged to a base-0 tile: partition_broadcast reads partition 0)
                dnb = pn.tile([1, 1024], F32, name="dnb", tag="dnb")
                nc.vector.tensor_copy(dnb[:], ytc[64:65, :])
                rbb = pn.tile([64, 1024], F32, name="rbb", tag="rbb")
                nc.gpsimd.partition_broadcast(rbb[:], dnb[:])
                rbs = pn.tile([64, 1024], F32, name="rbs", tag="rbs")
                nc.vector.reciprocal_approx_fast(rbs[:], rbb[:])
                # normalize: yn0 rows = [h0 | h2], yn1 rows = [h1 | h3]
                # (upper halves staged at base 0 then DMA-moved: walrus requires
                # tensor_tensor dst/src start partitions to match)
                for m in range(2):
                    nc.vector.tensor_mul(yn[m][0:64, sq],
                                         ytc[0:64, 512 * m:512 * m + 256],
                                         rbs[:, 512 * m:512 * m + 256])
                    stg = pa.tile([64, 256], BF16, name="stg", tag="stg", bufs=2)
                    nc.vector.tensor_mul(stg[:],
                                         ytc[0:64, 512 * m + 256:512 * m + 512],
                                         rbs[:, 512 * m + 256:512 * m + 512])
                    nc.sync.dma_start(yn[m][64:128, sq], stg[:])

                # output projection for the two 128-row s-tiles of this block
                for t in (2 * b, 2 * b + 1):
                    ssl = slice(128 * t, 128 * (t + 1))
                    pot = apools["po"].tile([128, 1024], F32, name="pot", tag="pot")
                    for nh in range(2):
                        nsl = slice(512 * nh, 512 * (nh + 1))
                        for kk in range(2):
                            nc.tensor.matmul(
                                pot[:, nsl], yn[kk][:, ssl],
                                wo[:, kk * D + 512 * nh:kk * D + 512 * (nh + 1)],
                                start=(kk == 0), stop=(kk == 1),
                                skip_group_check=(nh == 1))
                    ot = ob.tile([128, D], BF16, name="ot", tag="ot")
                    nc.vector.tensor_copy(ot[:], pot[:])
                    nc.sync.dma_start(out_d[ssl, :], ot[:])

            # ---- emission: all phase-1 chunks (own PSUM pool), then the
            # attention blocks with their own PSUM pools ----
            apools = {}
            for g in range(2):
                p1_proj(2 * g)
                p1_proj(2 * g + 1)
                p1_tail(g)
            pctx.close()
            apools["ps"] = pctx.enter_context(
                tc.tile_pool(name="ps", bufs=2, space=bass.MemorySpace.PSUM))
            apools["py"] = pctx.enter_context(
                tc.tile_pool(name="py", bufs=1, space=bass.MemorySpace.PSUM))
            apools["po"] = pctx.enter_context(
                tc.tile_pool(name="po", bufs=1, space=bass.MemorySpace.PSUM))
            for b in range(NB):
                attn_block(b)

    nc.finalize()
    return nc


_NC = None


def _get_nc():
    global _NC
    if _NC is None:
        _NC = _build()
    return _NC


def _perm():
    tops = [h * 64 + i for h in range(HG) for i in range(32)]
    bots = [h * 64 + 32 + i for h in range(HG) for i in range(32)]
    return tops + bots


def build_inmaps(x, Wq, Wk, Wv, Wo, q_gain):
    x = np.asarray(x, dtype=np.float32)
    Wq = np.asarray(Wq, dtype=np.float32)
    Wk = np.asarray(Wk, dtype=np.float32)
    Wv = np.asarray(Wv, dtype=np.float32)
    Wo = np.asarray(Wo, dtype=np.float32)
    q_gain = np.asarray(q_gain, dtype=np.float32)

    perm = _perm()
    in_maps = []
    for c in range(8):
        dp, tp = divmod(c, 4)
        # xt[p, k*S+s] = x[dp][s, 128k+p]
        xt_p = np.ascontiguousarray(
            x[dp].reshape(S, NK, 128).transpose(2, 1, 0).reshape(128, NK * S)
        ).astype(BF16NP)
        wq_sel = Wq[tp * E:(tp + 1) * E].T[:, perm]          # [D, 256] permuted
        wq_p = np.ascontiguousarray(
            wq_sel.reshape(NK, 128, E).transpose(1, 0, 2).reshape(128, NK * E)
        ).astype(BF16NP)
        wk_sel = Wk[tp * HD:(tp + 1) * HD].T                  # [D, 64]
        wv_sel = Wv[tp * HD:(tp + 1) * HD].T
        wkv_sel = np.concatenate([wk_sel, wv_sel], axis=1)    # [D, 128]
        wkv_p = np.ascontiguousarray(
            wkv_sel.reshape(NK, 128, 128).transpose(1, 0, 2).reshape(128, NK * 128)
        ).astype(BF16NP)
        # wo rows ordered [h0, h2, h1, h3] to match yn stacking
        horder = [0, 2, 1, 3]
        wo_cols = np.concatenate(
            [np.arange(tp * E + h * HD, tp * E + (h + 1) * HD) for h in horder])
        wo_sel = Wo[:, wo_cols].T                             # [256, D]
        wo_p = np.ascontiguousarray(
            wo_sel.reshape(2, 128, D).transpose(1, 0, 2).reshape(128, 2 * D)
        ).astype(BF16NP)
        g = q_gain[tp * HG:(tp + 1) * HG].astype(np.float64)
        qg8 = (g / 8.0).astype(np.float32).reshape(4, 1)
        in_maps.append({
            "xt": xt_p, "wq": wq_p, "wkv": wkv_p, "wo": wo_p, "qg8": qg8,
        })
    return in_maps


def kernel(x, Wq, Wk, Wv, Wo, q_gain):
    in_maps = build_inmaps(x, Wq, Wk, Wv, Wo, q_gain)
    nc = _get_nc()
    res = run_bass_kernel_spmd(nc, in_maps, core_ids=list(range(8)))
    out = np.zeros((B, S, D), dtype=np.float32)
    for c in range(8):
        out[c // 4] += res.results[c]["out"].astype(np.float32)
    return out
